# revision 23
# baseline (speedup 1.0000x reference)
"""Trainium2 Bass kernel for nn_DecoderConv (WeightedConv1D + BatchNorm + ReLU).

  out[b,o,l] = relu(BN_{B,L}(sum_{c,k} W[o,c,k] * w[b,k,l] * x[b,c,l+k-4]))
  w[b,k,l]   = exp(-||coords[b,:,l+k-4]-coords[b,:,l]||^2 / 2)

Sharding: sequence-parallel over L across 8 NeuronCores; halos are added
host-side (x +-4, coords +-8) so no inter-core exchange is needed except a
[128,2] AllReduce of the BatchNorm statistics (DRAM bounce buffers).

Key structure (per core, Lsh=16384):
  * Gaussian symmetry w[b,k,l] = w[b,8-k,l+k-4]: only taps k'=0..3 are
    computed (k=4 is exactly 1); mirrored taps are shifted views.
  * Unshifted products P_m[c,j] = x_pad[c,j]*w[m,j-4] satisfy
    R_k[c,l] = P_{8-k}[c,l+k], so all 9 conv taps become plain shifted-AP
    matmuls over 8 product tiles + the raw x tile.
  * dist2 via two overlapping-window DMAs ([48,C] tap-stacked views of
    coords) + one DVE subtract + one DVE square; the d-sum is a tiny
    selector matmul; exp(-d2/2) rides the ACT Exp scale.
  * The per-column weight broadcast across 64 channels (impossible on DVE:
    no partition-broadcast operand) is done by tiny selector matmuls
    E[16,128]^T @ w9[16,T] -> PSUM; DVE tensor_tensor (fp32 1x) forms P_m.
  * All matmuls use float32r (1 cycle/row vs 4 for fp32; ~2e-4 rel err).
  * Batch pairs are packed as 128 = 2x64 partitions; the 18 accumulating
    conv matmuls per tile alternate 64-row groups (row-tiling concurrency).
  * conv_out (16.8 MB) stays SBUF-resident between the conv pass and the
    normalize pass - no second HBM round trip. ACT copies PSUM->SBUF with
    fused accum_out channel sums; an ACT Square pass accumulates sum-sq.
  * rstd = exp(-0.5*ln(var+eps)) keeps everything in one ACT table set
    (natural_log_exp_and_others: Exp/Ln/Copy/Square/Relu - no reloads).
  * Final ACT Relu(scale*x+bias) streams conv_out to HBM.
  * Main loop is software-pipelined one tile ahead (products for tile i+1
    are emitted between the conv matmul halves of tile i).

Cost-model timeline: ~477 us/core (DVE-bound: the 8 weighted-product
tensor_tensor passes are the floor; fp32 TT runs at 1 elem/lane/cycle).
conv bias is dropped: it cancels exactly through training-mode BN.
"""

import math

import numpy as np

# problem sizes (hardcoded per contract)
B, CIN, COUT, LFULL = 4, 64, 64, 131072
KK, PAD = 9, 4
NCORES = 8
SIGMA = 1.0
EPS = 1e-5

_CACHE = {}


def _trace(nc, tile, mybir, L, n_cores):
    """Emit the whole program for one core under a TileContext."""
    Lsh = L // n_cores
    TL = 504                       # output columns per tile
    NT = math.ceil(Lsh / TL)
    C1 = 1024                      # w9-phase chunk
    NC1 = math.ceil((Lsh + 8) / C1)
    CN = min(2048, Lsh)            # normalize-phase chunk
    f32 = mybir.dt.float32
    f32r = mybir.dt.float32r
    Alu = mybir.AluOpType
    Act = mybir.ActivationFunctionType
    MLIST = [0, 1, 2, 3, 5, 6, 7, 8]

    x_t = nc.dram_tensor("xsh", [B, CIN, Lsh + 8], f32r, kind="ExternalInput")
    cp_t = nc.dram_tensor("cpsh", [B, 3, Lsh + 16], f32, kind="ExternalInput")
    wst_t = nc.dram_tensor("wstack", [128, KK * COUT], f32r, kind="ExternalInput")
    e16_t = nc.dram_tensor("e16", [16, 2 * 8 * 128], f32r, kind="ExternalInput")
    s48_t = nc.dram_tensor("s48", [48, 16], f32r, kind="ExternalInput")
    fold_t = nc.dram_tensor("fold", [128, 128], f32, kind="ExternalInput")
    gb_t = nc.dram_tensor("gb", [128, 2], f32, kind="ExternalInput")
    out_t = nc.dram_tensor("outsh", [B, COUT, Lsh], f32, kind="ExternalOutput")

    with tile.TileContext(nc) as tc:
        with tc.tile_pool(name="consts", bufs=1) as cpool, \
             tc.tile_pool(name="convout", bufs=1) as opool, \
             tc.tile_pool(name="acc", bufs=1) as apool, \
             tc.tile_pool(name="dram", bufs=1, space="DRAM") as dpool:

            wst = cpool.tile([128, KK * COUT], f32r, name="wst")
            e16 = cpool.tile([16, 2 * 8 * 128], f32r, name="e16c")
            s48 = cpool.tile([48, 16], f32r, name="s48c")
            fold = cpool.tile([128, 128], f32, name="foldc")
            gb = cpool.tile([128, 2], f32, name="gbc")
            nc.sync.dma_start(wst[:], wst_t.ap())
            nc.sync.dma_start(e16[:], e16_t.ap())
            nc.sync.dma_start(s48[:], s48_t.ap())
            nc.sync.dma_start(fold[:], fold_t.ap())
            nc.sync.dma_start(gb[:], gb_t.ap())

            conv_out = opool.tile([128, 2 * Lsh], f32, name="convout")
            accS = apool.tile([128, NT], f32, name="accS")
            accSS = apool.tile([128, NT], f32, name="accSS")

            w9e_d = dpool.tile([16, Lsh + 16], f32r, name="w9ed")
            bnc_in = dpool.tile([128, 2], f32, name="bncin")
            bnc_out = dpool.tile([128, 2], f32, name="bncout")

            cp_flat = cp_t.ap().rearrange("b d i -> (b d) i")  # rows (b,d)=b*3+d

            # zero tail of w9e [Lsh+8, Lsh+16) before anything reads it
            ztail = cpool.tile([16, 8], f32, name="ztail")
            nc.vector.memset(ztail[:], 0.0)
            nc.gpsimd.dma_start(w9e_d[:, Lsh + 8 :], ztail[:])

            # ---------------- phase 1: w9e (Gaussian weights, taps 0..3) ---------
            # ct9s[(kp,row), i] = cp[row, i0+i+kp] via one DMA with an
            # overlapping-window 3D source AP; ct9u = cp[row, i0+i+4] repeated
            # (0-step leading dim).
            import concourse.bass as _bass
            cp_th = cp_t  # tensor handle
            row_stride = Lsh + 16
            with tc.tile_pool(name="ph1", bufs=2) as p1, \
                 tc.tile_pool(name="ph1ps", bufs=4, space="PSUM") as p1ps:
                for ci in range(NC1):
                    i0 = ci * C1
                    cw = min(C1, Lsh + 8 - i0)
                    ct9s = p1.tile([48, C1], f32, tag="ct9s")
                    ct9u = p1.tile([48, C1], f32, tag="ct9u")
                    src_s = _bass.AP(cp_th, i0, [[1, 4], [row_stride, 12], [1, cw]])
                    src_u = _bass.AP(cp_th, i0 + 4, [[0, 4], [row_stride, 12], [1, cw]])
                    nc.sync.dma_start(ct9s[:, :cw], src_s)
                    nc.scalar.dma_start(ct9u[:, :cw], src_u)
                    nc.vector.tensor_tensor(
                        ct9s[:, :cw], ct9s[:, :cw], ct9u[:, :cw], Alu.subtract
                    )
                    d2s = p1.tile([48, C1], f32r, tag="d2s")
                    nc.vector.tensor_tensor(
                        d2s[:, :cw], ct9s[:, :cw], ct9s[:, :cw], Alu.mult
                    )
                    w9c = p1.tile([16, C1], f32r, tag="w9c")
                    for n0 in range(0, cw, 512):
                        nw = min(512, cw - n0)
                        ps = p1ps.tile([16, 512], f32, tag="w9ps")
                        nc.tensor.matmul(
                            ps[:, :nw], s48[:],
                            d2s[:, n0 : n0 + nw],
                            start=True, stop=True,
                        )
                        nc.scalar.activation(
                            w9c[:, n0 : n0 + nw], ps[:, :nw], Act.Exp, scale=-0.5
                        )
                    nc.sync.dma_start(w9e_d[:, i0 : i0 + cw], w9c[:, :cw])

            # ---------------- phase 2: conv main loop ---------------------------
            # software-pipelined: products (PE wbc + DVE mult) for unit i+1 are
            # emitted before unit i's conv matmuls so DVE overlaps PE.
            x_pair = x_t.ap().rearrange("(pr par) c i -> pr (par c) i", par=2)
            with tc.tile_pool(name="main", bufs=3) as mp, \
                 tc.tile_pool(name="ppool", bufs=18) as ppool, \
                 tc.tile_pool(name="sqp", bufs=2) as sqp, \
                 tc.tile_pool(name="wbcps", bufs=4, space="PSUM") as wbcps, \
                 tc.tile_pool(name="convps", bufs=2, space="PSUM") as convps:
                units = [(p, t) for p in range(2) for t in range(NT)]

                def load_unit(u):
                    p, t = u
                    l0 = t * TL
                    tl = min(TL, Lsh - l0)
                    tw = tl + 8
                    xt = mp.tile([128, TL + 8], f32r, tag="xt")
                    nc.sync.dma_start(xt[:, :tw], x_pair[p, :, l0 : l0 + tw])
                    w9a = mp.tile([16, TL + 12], f32r, tag="w9a")
                    nc.sync.dma_start(w9a[:, : tw + 4], w9e_d[:, l0 : l0 + tw + 4])
                    return (p, t, tl, l0, tw, xt, w9a)

                def products(st_u, half):
                    p, t, tl, l0, tw, xt, w9a = st_u
                    pp = {}
                    for mi, m in enumerate(MLIST):
                        if (mi < 4) != (half == 0):
                            continue
                        wbc = wbcps.tile([128, 512], f32, tag="wbc")
                        off = 0 if m <= 3 else m - 4
                        nc.tensor.matmul(
                            wbc[:, :tw],
                            e16[:, (p * 8 + mi) * 128 : (p * 8 + mi + 1) * 128],
                            w9a[:, off : off + tw],
                            start=True, stop=True,
                        )
                        pt = ppool.tile([128, TL + 8], f32r, tag="pp")
                        nc.vector.tensor_tensor(
                            pt[:, :tw], xt[:, :tw].bitcast(f32), wbc[:, :tw],
                            Alu.mult,
                        )
                        pp[m] = pt[:]
                    return pp

                st = [None, None]   # current, next
                pp_cur = {}
                st[0] = load_unit(units[0])
                pp_cur = products(st[0], 0)
                pp_cur.update(products(st[0], 1))
                pp_cur[4] = st[0][5][:]
                for i in range(len(units)):
                    pp_next = {}
                    if i + 1 < len(units):
                        st[1] = load_unit(units[i + 1])
                        pp_next = products(st[1], 0)
                    # conv for current unit, first half
                    p, t, tl, l0, tw, xt, w9a = st[0]
                    cps = convps.tile([64, 1024], f32, tag="cps")
                    for k in range(KK):
                        if k == 5 and i + 1 < len(units):
                            pp_next.update(products(st[1], 1))
                            pp_next[4] = st[1][5][:]
                        src = pp_cur[8 - k]
                        for bi in range(2):
                            nc.tensor.matmul(
                                cps[:, 512 * bi : 512 * bi + tl],
                                wst[64 * bi : 64 * bi + 64, k * 64 : (k + 1) * 64],
                                src[64 * bi : 64 * bi + 64, k : k + tl],
                                start=(k == 0), stop=(k == KK - 1),
                            )
                    cps_v = cps[:].rearrange("p (two n) -> p two n", two=2)[:, :, :tl]
                    co_v = conv_out[64 * p : 64 * p + 64].rearrange(
                        "p (two n) -> p two n", two=2
                    )[:, :, l0 : l0 + tl]
                    nc.scalar.activation(
                        co_v, cps_v, Act.Copy,
                        accum_out=accS[64 * p : 64 * p + 64, t : t + 1],
                    )
                    sq = sqp.tile([64, 1024], f32, tag="sq")
                    sq_v = sq[:].rearrange("p (two n) -> p two n", two=2)[:, :, :tl]
                    nc.scalar.activation(
                        sq_v, co_v, Act.Square,
                        accum_out=accSS[64 * p : 64 * p + 64, t : t + 1],
                    )
                    st[0] = st[1]
                    pp_cur = pp_next

            # ---------------- phase 3: BN stats + all-reduce ---------------------
            with tc.tile_pool(name="stats", bufs=1) as sp, \
                 tc.tile_pool(name="statps", bufs=1, space="PSUM") as spps:
                st = sp.tile([128, 2], f32, name="st")
                nc.vector.tensor_reduce(
                    st[:, 0:1], accS[:], mybir.AxisListType.X, Alu.add
                )
                nc.vector.tensor_reduce(
                    st[:, 1:2], accSS[:], mybir.AxisListType.X, Alu.add
                )
                if n_cores > 1:
                    nc.gpsimd.dma_start(bnc_in[:], st[:])
                    nc.gpsimd.collective_compute(
                        "AllReduce",
                        Alu.add,
                        replica_groups=[list(range(n_cores))],
                        ins=[bnc_in.opt()],
                        outs=[bnc_out.opt()],
                    )
                    stR = sp.tile([128, 2], f32, name="stR")
                    nc.sync.dma_start(stR[:], bnc_out[:])
                else:
                    stR = st
                fps = spps.tile([128, 2], f32, name="fps")
                nc.tensor.matmul(fps[:], fold[:], stR[:], start=True, stop=True)
                n_inv = 1.0 / float(B * L)
                mean = sp.tile([128, 1], f32, name="mean")
                nc.vector.tensor_scalar_mul(mean[:], fps[:, 0:1], n_inv)
                m2n = sp.tile([128, 1], f32, name="m2n")
                nc.vector.tensor_scalar(
                    m2n[:], mean[:], mean[:], -1.0, Alu.mult, Alu.mult
                )
                var = sp.tile([128, 1], f32, name="var")
                nc.vector.tensor_scalar(
                    var[:], fps[:, 1:2], n_inv, EPS, Alu.mult, Alu.add
                )
                nc.vector.tensor_tensor(var[:], var[:], m2n[:], Alu.add)
                lnv = sp.tile([128, 1], f32, name="lnv")
                nc.scalar.activation(lnv[:], var[:], Act.Ln)
                rstd = sp.tile([128, 1], f32, name="rstd")
                nc.scalar.activation(rstd[:], lnv[:], Act.Exp, scale=-0.5)
                scl = sp.tile([128, 1], f32, name="scl")
                nc.vector.tensor_tensor(scl[:], gb[:, 0:1], rstd[:], Alu.mult)
                mscl = sp.tile([128, 1], f32, name="mscl")
                nc.vector.tensor_tensor(mscl[:], mean[:], scl[:], Alu.mult)
                bia = sp.tile([128, 1], f32, name="bia")
                nc.vector.tensor_tensor(bia[:], gb[:, 1:2], mscl[:], Alu.subtract)

                # ------------ phase 4: normalize + relu + store -----------------
                out_par = out_t.ap().rearrange(
                    "(pr par) o i -> par pr o i", par=2
                )
                with tc.tile_pool(name="norm", bufs=3) as np_:
                    for c0 in range(0, 2 * Lsh, CN):
                        parity = c0 // Lsh
                        lr = c0 % Lsh
                        nst = np_.tile([128, CN], f32, tag="nst")
                        nc.scalar.activation(
                            nst[:], conv_out[:, c0 : c0 + CN], Act.Relu,
                            bias=bia[:], scale=scl[:],
                        )
                        nc.sync.dma_start(
                            out_par[parity, :, :, lr : lr + CN], nst[:]
                        )
    return nc


def build_program(L=LFULL, n_cores=NCORES, debug=False):
    from concourse import bacc
    import concourse.tile as tile
    import concourse.mybir as mybir

    nc = bacc.Bacc(
        "TRN2",
        target_bir_lowering=False,
        debug=debug,
        enable_asserts=False,
        num_devices=n_cores,
    )
    _trace(nc, tile, mybir, L, n_cores)
    nc.compile()
    return nc


def make_consts(W):
    """Host-side constant tensors."""
    W = np.asarray(W, np.float32)
    wstack = np.zeros((128, KK * COUT), np.float32)
    for r in range(128):
        for k in range(KK):
            wstack[r, k * 64 : (k + 1) * 64] = W[:, r % 64, k]
    e16 = np.zeros((16, 2 * 8 * 128), np.float32)
    MLIST = [0, 1, 2, 3, 5, 6, 7, 8]
    for p2 in range(2):
        for mi, m in enumerate(MLIST):
            kk = m if m <= 3 else 8 - m
            base = (p2 * 8 + mi) * 128
            e16[kk * 4 + 2 * p2, base : base + 64] = 1.0
            e16[kk * 4 + 2 * p2 + 1, base + 64 : base + 128] = 1.0
    s48 = np.zeros((48, 16), np.float32)
    for kp in range(4):
        for b in range(B):
            for d in range(3):
                s48[12 * kp + b * 3 + d, kp * 4 + b] = 1.0
    fold = np.zeros((128, 128), np.float32)
    for j in range(128):
        fold[j, j % 64] = 1.0
        fold[j, j % 64 + 64] = 1.0
    return wstack, e16, s48, fold


def shard_inputs(x, coords, W, gamma, beta, L=LFULL, n_cores=NCORES):
    x = np.ascontiguousarray(np.asarray(x, np.float32))
    coords = np.ascontiguousarray(np.asarray(coords, np.float32))
    Lsh = L // n_cores
    xp = np.pad(x, ((0, 0), (0, 0), (PAD, PAD)))
    cp = np.pad(coords, ((0, 0), (0, 0), (2 * PAD, 2 * PAD)))
    wstack, e16, s48, fold = make_consts(W)
    gb = np.stack(
        [np.tile(np.asarray(gamma, np.float32), 2),
         np.tile(np.asarray(beta, np.float32), 2)], axis=1
    ).astype(np.float32)
    in_maps = []
    for c in range(n_cores):
        s = c * Lsh
        in_maps.append({
            "xsh": np.ascontiguousarray(xp[:, :, s : s + Lsh + 8]),
            "cpsh": np.ascontiguousarray(cp[:, :, s : s + Lsh + 16]),
            "wstack": wstack,
            "e16": e16,
            "s48": s48,
            "fold": fold,
            "gb": gb,
        })
    return in_maps


def assemble(results, L=LFULL, n_cores=NCORES):
    Lsh = L // n_cores
    out = np.empty((B, COUT, L), np.float32)
    for c in range(n_cores):
        out[:, :, c * Lsh : (c + 1) * Lsh] = results[c]["outsh"]
    return out


def kernel(x, coords, W, bias, gamma, beta):
    from concourse import bass_utils

    key = "full"
    if key not in _CACHE:
        _CACHE[key] = build_program()
    nc = _CACHE[key]
    in_maps = shard_inputs(x, coords, W, gamma, beta)
    res = bass_utils.run_bass_kernel_spmd(
        nc, in_maps, core_ids=list(range(NCORES))
    )
    return assemble(res.results)


if __name__ == "__main__":
    d = np.load("/tmp/refdata.npz")
    got = kernel(d["x"], d["coords"], d["W"], d["bias"], d["gamma"], d["beta"])
    exp = d["expected"]
    err = np.abs(got - exp).max()
    print("abs err:", err, "rel:", err / np.abs(exp).max())


# revision 26
# speedup vs baseline: 1.0027x; 1.0027x over previous
"""Trainium2 Bass kernel for nn_DecoderConv (WeightedConv1D + BatchNorm + ReLU).

  out[b,o,l] = relu(BN_{B,L}(sum_{c,k} W[o,c,k] * w[b,k,l] * x[b,c,l+k-4]))
  w[b,k,l]   = exp(-||coords[b,:,l+k-4]-coords[b,:,l]||^2 / 2)

Sharding: sequence-parallel over L across 8 NeuronCores; halos are added
host-side (x +-4, coords +-8) so no inter-core exchange is needed except a
[128,2] AllReduce of the BatchNorm statistics (DRAM bounce buffers).

Key structure (per core, Lsh=16384):
  * Gaussian symmetry w[b,k,l] = w[b,8-k,l+k-4]: only taps k'=0..3 are
    computed (k=4 is exactly 1); mirrored taps are shifted views.
  * Unshifted products P_m[c,j] = x_pad[c,j]*w[m,j-4] satisfy
    R_k[c,l] = P_{8-k}[c,l+k], so all 9 conv taps become plain shifted-AP
    matmuls over 8 product tiles + the raw x tile.
  * dist2 via two overlapping-window DMAs ([48,C] tap-stacked views of
    coords) + one DVE subtract + one DVE square; the d-sum is a tiny
    selector matmul; exp(-d2/2) rides the ACT Exp scale.
  * The per-column weight broadcast across 64 channels (impossible on DVE:
    no partition-broadcast operand) is done by tiny selector matmuls
    E[16,128]^T @ w9[16,T] -> PSUM; DVE tensor_tensor (fp32 1x) forms P_m.
  * All matmuls use float32r (1 cycle/row vs 4 for fp32; ~2e-4 rel err).
  * Batch pairs are packed as 128 = 2x64 partitions; the 18 accumulating
    conv matmuls per tile alternate 64-row groups (row-tiling concurrency).
  * conv_out (16.8 MB) stays SBUF-resident between the conv pass and the
    normalize pass - no second HBM round trip. ACT copies PSUM->SBUF with
    fused accum_out channel sums; an ACT Square pass accumulates sum-sq.
  * rstd = exp(-0.5*ln(var+eps)) keeps everything in one ACT table set
    (natural_log_exp_and_others: Exp/Ln/Copy/Square/Relu - no reloads).
  * Final ACT Relu(scale*x+bias) streams conv_out to HBM.
  * Main loop is software-pipelined one tile ahead (products for tile i+1
    are emitted between the conv matmul halves of tile i).

Cost-model timeline: ~477 us/core (DVE-bound: the 8 weighted-product
tensor_tensor passes are the floor; fp32 TT runs at 1 elem/lane/cycle).
conv bias is dropped: it cancels exactly through training-mode BN.
"""

import math

import numpy as np

# problem sizes (hardcoded per contract)
B, CIN, COUT, LFULL = 4, 64, 64, 131072
KK, PAD = 9, 4
NCORES = 8
SIGMA = 1.0
EPS = 1e-5

_CACHE = {}


def _trace(nc, tile, mybir, L, n_cores):
    """Emit the whole program for one core under a TileContext."""
    Lsh = L // n_cores
    TL = 504                       # output columns per tile
    NT = math.ceil(Lsh / TL)
    C1 = 1024                      # w9-phase chunk
    NC1 = math.ceil((Lsh + 8) / C1)
    CN = min(2048, Lsh)            # normalize-phase chunk
    f32 = mybir.dt.float32
    f32r = mybir.dt.float32r
    Alu = mybir.AluOpType
    Act = mybir.ActivationFunctionType
    MLIST = [0, 1, 2, 3, 5, 6, 7, 8]

    x_t = nc.dram_tensor("xsh", [B, CIN, Lsh + 8], f32r, kind="ExternalInput")
    cp_t = nc.dram_tensor("cpsh", [B, 3, Lsh + 16], f32, kind="ExternalInput")
    wst_t = nc.dram_tensor("wstack", [128, KK * COUT], f32r, kind="ExternalInput")
    e16_t = nc.dram_tensor("e16", [16, 2 * 8 * 128], f32r, kind="ExternalInput")
    s48_t = nc.dram_tensor("s48", [48, 16], f32r, kind="ExternalInput")
    fold_t = nc.dram_tensor("fold", [128, 128], f32, kind="ExternalInput")
    gb_t = nc.dram_tensor("gb", [128, 2], f32, kind="ExternalInput")
    out_t = nc.dram_tensor("outsh", [B, COUT, Lsh], f32, kind="ExternalOutput")

    with tile.TileContext(nc) as tc:
        with tc.tile_pool(name="consts", bufs=1) as cpool, \
             tc.tile_pool(name="convout", bufs=1) as opool, \
             tc.tile_pool(name="acc", bufs=1) as apool, \
             tc.tile_pool(name="dram", bufs=1, space="DRAM") as dpool:

            wst = cpool.tile([128, KK * COUT], f32r, name="wst")
            e16 = cpool.tile([16, 2 * 8 * 128], f32r, name="e16c")
            s48 = cpool.tile([48, 16], f32r, name="s48c")
            fold = cpool.tile([128, 128], f32, name="foldc")
            gb = cpool.tile([128, 2], f32, name="gbc")
            nc.sync.dma_start(wst[:], wst_t.ap())
            nc.sync.dma_start(e16[:], e16_t.ap())
            nc.sync.dma_start(s48[:], s48_t.ap())
            nc.sync.dma_start(fold[:], fold_t.ap())
            nc.sync.dma_start(gb[:], gb_t.ap())

            conv_out = opool.tile([128, 2 * Lsh], f32, name="convout")
            accS = apool.tile([128, NT], f32, name="accS")
            accSS = apool.tile([128, NT], f32, name="accSS")

            w9e_d = dpool.tile([16, Lsh + 16], f32r, name="w9ed")
            bnc_in = dpool.tile([128, 2], f32, name="bncin")
            bnc_out = dpool.tile([128, 2], f32, name="bncout")

            cp_flat = cp_t.ap().rearrange("b d i -> (b d) i")  # rows (b,d)=b*3+d

            # zero tail of w9e [Lsh+8, Lsh+16) before anything reads it
            ztail = cpool.tile([16, 8], f32, name="ztail")
            nc.vector.memset(ztail[:], 0.0)
            nc.gpsimd.dma_start(w9e_d[:, Lsh + 8 :], ztail[:])

            # ---------------- phase 1: w9e (Gaussian weights, taps 0..3) ---------
            # ct9s[(kp,row), i] = cp[row, i0+i+kp] via one DMA with an
            # overlapping-window 3D source AP; ct9u = cp[row, i0+i+4] repeated
            # (0-step leading dim).
            import concourse.bass as _bass
            cp_th = cp_t  # tensor handle
            row_stride = Lsh + 16
            with tc.tile_pool(name="ph1", bufs=2) as p1, \
                 tc.tile_pool(name="ph1ps", bufs=4, space="PSUM") as p1ps:
                for ci in range(NC1):
                    i0 = ci * C1
                    cw = min(C1, Lsh + 8 - i0)
                    ct9s = p1.tile([48, C1], f32, tag="ct9s")
                    ct9u = p1.tile([48, C1], f32, tag="ct9u")
                    src_s = _bass.AP(cp_th, i0, [[1, 4], [row_stride, 12], [1, cw]])
                    src_u = _bass.AP(cp_th, i0 + 4, [[0, 4], [row_stride, 12], [1, cw]])
                    nc.sync.dma_start(ct9s[:, :cw], src_s)
                    nc.scalar.dma_start(ct9u[:, :cw], src_u)
                    nc.vector.tensor_tensor(
                        ct9s[:, :cw], ct9s[:, :cw], ct9u[:, :cw], Alu.subtract
                    )
                    d2s = p1.tile([48, C1], f32r, tag="d2s")
                    nc.vector.tensor_tensor(
                        d2s[:, :cw], ct9s[:, :cw], ct9s[:, :cw], Alu.mult
                    )
                    w9c = p1.tile([16, C1], f32r, tag="w9c")
                    for n0 in range(0, cw, 512):
                        nw = min(512, cw - n0)
                        ps = p1ps.tile([16, 512], f32, tag="w9ps")
                        nc.tensor.matmul(
                            ps[:, :nw], s48[:],
                            d2s[:, n0 : n0 + nw],
                            start=True, stop=True,
                        )
                        nc.scalar.activation(
                            w9c[:, n0 : n0 + nw], ps[:, :nw], Act.Exp, scale=-0.5
                        )
                    nc.sync.dma_start(w9e_d[:, i0 : i0 + cw], w9c[:, :cw])

            # ---------------- phase 2: conv main loop ---------------------------
            # software-pipelined: products (PE wbc + DVE mult) for unit i+1 are
            # emitted before unit i's conv matmuls so DVE overlaps PE.
            x_pair = x_t.ap().rearrange("(pr par) c i -> pr (par c) i", par=2)
            with tc.tile_pool(name="main", bufs=3) as mp, \
                 tc.tile_pool(name="ppool", bufs=18) as ppool, \
                 tc.tile_pool(name="sqp", bufs=2) as sqp, \
                 tc.tile_pool(name="wbcps", bufs=4, space="PSUM") as wbcps, \
                 tc.tile_pool(name="convps", bufs=2, space="PSUM") as convps:
                units = [(p, t) for t in range(NT) for p in range(2)]

                def load_unit(u):
                    p, t = u
                    l0 = t * TL
                    tl = min(TL, Lsh - l0)
                    tw = tl + 8
                    xt = mp.tile([128, TL + 8], f32r, tag="xt")
                    nc.sync.dma_start(xt[:, :tw], x_pair[p, :, l0 : l0 + tw])
                    w9a = mp.tile([16, TL + 12], f32r, tag="w9a")
                    nc.sync.dma_start(w9a[:, : tw + 4], w9e_d[:, l0 : l0 + tw + 4])
                    return (p, t, tl, l0, tw, xt, w9a)

                def products(st_u, half):
                    p, t, tl, l0, tw, xt, w9a = st_u
                    pp = {}
                    for mi, m in enumerate(MLIST):
                        if (mi < 4) != (half == 0):
                            continue
                        wbc = wbcps.tile([128, 512], f32, tag="wbc")
                        off = 0 if m <= 3 else m - 4
                        nc.tensor.matmul(
                            wbc[:, :tw],
                            e16[:, (p * 8 + mi) * 128 : (p * 8 + mi + 1) * 128],
                            w9a[:, off : off + tw],
                            start=True, stop=True,
                        )
                        pt = ppool.tile([128, TL + 8], f32r, tag="pp")
                        nc.vector.tensor_tensor(
                            pt[:, :tw], xt[:, :tw].bitcast(f32), wbc[:, :tw],
                            Alu.mult,
                        )
                        pp[m] = pt[:]
                    return pp

                st = [None, None]   # current, next
                pp_cur = {}
                st[0] = load_unit(units[0])
                pp_cur = products(st[0], 0)
                pp_cur.update(products(st[0], 1))
                pp_cur[4] = st[0][5][:]
                for i in range(len(units)):
                    pp_next = {}
                    if i + 1 < len(units):
                        st[1] = load_unit(units[i + 1])
                        pp_next = products(st[1], 0)
                    # conv for current unit, first half
                    p, t, tl, l0, tw, xt, w9a = st[0]
                    cps = convps.tile([64, 1024], f32, tag="cps")
                    for k in range(KK):
                        if k == 5 and i + 1 < len(units):
                            pp_next.update(products(st[1], 1))
                            pp_next[4] = st[1][5][:]
                        src = pp_cur[8 - k]
                        for bi in range(2):
                            nc.tensor.matmul(
                                cps[:, 512 * bi : 512 * bi + tl],
                                wst[64 * bi : 64 * bi + 64, k * 64 : (k + 1) * 64],
                                src[64 * bi : 64 * bi + 64, k : k + tl],
                                start=(k == 0), stop=(k == KK - 1),
                            )
                    cps_v = cps[:].rearrange("p (two n) -> p two n", two=2)[:, :, :tl]
                    co_v = conv_out[64 * p : 64 * p + 64].rearrange(
                        "p (two n) -> p two n", two=2
                    )[:, :, l0 : l0 + tl]
                    nc.scalar.activation(
                        co_v, cps_v, Act.Copy,
                        accum_out=accS[64 * p : 64 * p + 64, t : t + 1],
                    )
                    sq = sqp.tile([64, 1024], f32, tag="sq")
                    sq_v = sq[:].rearrange("p (two n) -> p two n", two=2)[:, :, :tl]
                    nc.scalar.activation(
                        sq_v, co_v, Act.Square,
                        accum_out=accSS[64 * p : 64 * p + 64, t : t + 1],
                    )
                    st[0] = st[1]
                    pp_cur = pp_next

            # ---------------- phase 3: BN stats + all-reduce ---------------------
            with tc.tile_pool(name="stats", bufs=1) as sp, \
                 tc.tile_pool(name="statps", bufs=1, space="PSUM") as spps:
                st = sp.tile([128, 2], f32, name="st")
                nc.vector.tensor_reduce(
                    st[:, 0:1], accS[:], mybir.AxisListType.X, Alu.add
                )
                nc.vector.tensor_reduce(
                    st[:, 1:2], accSS[:], mybir.AxisListType.X, Alu.add
                )
                if n_cores > 1:
                    nc.gpsimd.dma_start(bnc_in[:], st[:])
                    nc.gpsimd.collective_compute(
                        "AllReduce",
                        Alu.add,
                        replica_groups=[list(range(n_cores))],
                        ins=[bnc_in.opt()],
                        outs=[bnc_out.opt()],
                    )
                    stR = sp.tile([128, 2], f32, name="stR")
                    nc.sync.dma_start(stR[:], bnc_out[:])
                else:
                    stR = st
                fps = spps.tile([128, 2], f32, name="fps")
                nc.tensor.matmul(fps[:], fold[:], stR[:], start=True, stop=True)
                n_inv = 1.0 / float(B * L)
                mean = sp.tile([128, 1], f32, name="mean")
                nc.vector.tensor_scalar_mul(mean[:], fps[:, 0:1], n_inv)
                m2n = sp.tile([128, 1], f32, name="m2n")
                nc.vector.tensor_scalar(
                    m2n[:], mean[:], mean[:], -1.0, Alu.mult, Alu.mult
                )
                var = sp.tile([128, 1], f32, name="var")
                nc.vector.tensor_scalar(
                    var[:], fps[:, 1:2], n_inv, EPS, Alu.mult, Alu.add
                )
                nc.vector.tensor_tensor(var[:], var[:], m2n[:], Alu.add)
                lnv = sp.tile([128, 1], f32, name="lnv")
                nc.scalar.activation(lnv[:], var[:], Act.Ln)
                rstd = sp.tile([128, 1], f32, name="rstd")
                nc.scalar.activation(rstd[:], lnv[:], Act.Exp, scale=-0.5)
                scl = sp.tile([128, 1], f32, name="scl")
                nc.vector.tensor_tensor(scl[:], gb[:, 0:1], rstd[:], Alu.mult)
                mscl = sp.tile([128, 1], f32, name="mscl")
                nc.vector.tensor_tensor(mscl[:], mean[:], scl[:], Alu.mult)
                bia = sp.tile([128, 1], f32, name="bia")
                nc.vector.tensor_tensor(bia[:], gb[:, 1:2], mscl[:], Alu.subtract)

                # ------------ phase 4: normalize + relu + store -----------------
                out_par = out_t.ap().rearrange(
                    "(pr par) o i -> par pr o i", par=2
                )
                with tc.tile_pool(name="norm", bufs=3) as np_:
                    for c0 in range(0, 2 * Lsh, CN):
                        parity = c0 // Lsh
                        lr = c0 % Lsh
                        nst = np_.tile([128, CN], f32, tag="nst")
                        nc.scalar.activation(
                            nst[:], conv_out[:, c0 : c0 + CN], Act.Relu,
                            bias=bia[:], scale=scl[:],
                        )
                        nc.sync.dma_start(
                            out_par[parity, :, :, lr : lr + CN], nst[:]
                        )
    return nc


def build_program(L=LFULL, n_cores=NCORES, debug=False):
    from concourse import bacc
    import concourse.tile as tile
    import concourse.mybir as mybir

    nc = bacc.Bacc(
        "TRN2",
        target_bir_lowering=False,
        debug=debug,
        enable_asserts=False,
        num_devices=n_cores,
    )
    _trace(nc, tile, mybir, L, n_cores)
    nc.compile()
    return nc


def make_consts(W):
    """Host-side constant tensors."""
    W = np.asarray(W, np.float32)
    wstack = np.zeros((128, KK * COUT), np.float32)
    for r in range(128):
        for k in range(KK):
            wstack[r, k * 64 : (k + 1) * 64] = W[:, r % 64, k]
    e16 = np.zeros((16, 2 * 8 * 128), np.float32)
    MLIST = [0, 1, 2, 3, 5, 6, 7, 8]
    for p2 in range(2):
        for mi, m in enumerate(MLIST):
            kk = m if m <= 3 else 8 - m
            base = (p2 * 8 + mi) * 128
            e16[kk * 4 + 2 * p2, base : base + 64] = 1.0
            e16[kk * 4 + 2 * p2 + 1, base + 64 : base + 128] = 1.0
    s48 = np.zeros((48, 16), np.float32)
    for kp in range(4):
        for b in range(B):
            for d in range(3):
                s48[12 * kp + b * 3 + d, kp * 4 + b] = 1.0
    fold = np.zeros((128, 128), np.float32)
    for j in range(128):
        fold[j, j % 64] = 1.0
        fold[j, j % 64 + 64] = 1.0
    return wstack, e16, s48, fold


def shard_inputs(x, coords, W, gamma, beta, L=LFULL, n_cores=NCORES):
    x = np.ascontiguousarray(np.asarray(x, np.float32))
    coords = np.ascontiguousarray(np.asarray(coords, np.float32))
    Lsh = L // n_cores
    xp = np.pad(x, ((0, 0), (0, 0), (PAD, PAD)))
    cp = np.pad(coords, ((0, 0), (0, 0), (2 * PAD, 2 * PAD)))
    wstack, e16, s48, fold = make_consts(W)
    gb = np.stack(
        [np.tile(np.asarray(gamma, np.float32), 2),
         np.tile(np.asarray(beta, np.float32), 2)], axis=1
    ).astype(np.float32)
    in_maps = []
    for c in range(n_cores):
        s = c * Lsh
        in_maps.append({
            "xsh": np.ascontiguousarray(xp[:, :, s : s + Lsh + 8]),
            "cpsh": np.ascontiguousarray(cp[:, :, s : s + Lsh + 16]),
            "wstack": wstack,
            "e16": e16,
            "s48": s48,
            "fold": fold,
            "gb": gb,
        })
    return in_maps


def assemble(results, L=LFULL, n_cores=NCORES):
    Lsh = L // n_cores
    out = np.empty((B, COUT, L), np.float32)
    for c in range(n_cores):
        out[:, :, c * Lsh : (c + 1) * Lsh] = results[c]["outsh"]
    return out


def kernel(x, coords, W, bias, gamma, beta):
    from concourse import bass_utils

    key = "full"
    if key not in _CACHE:
        _CACHE[key] = build_program()
    nc = _CACHE[key]
    in_maps = shard_inputs(x, coords, W, gamma, beta)
    res = bass_utils.run_bass_kernel_spmd(
        nc, in_maps, core_ids=list(range(NCORES))
    )
    return assemble(res.results)


if __name__ == "__main__":
    d = np.load("/tmp/refdata.npz")
    got = kernel(d["x"], d["coords"], d["W"], d["bias"], d["gamma"], d["beta"])
    exp = d["expected"]
    err = np.abs(got - exp).max()
    print("abs err:", err, "rel:", err / np.abs(exp).max())
